# revision 95
# baseline (speedup 1.0000x reference)
"""Trainium2 Bass kernel for nn_DecoderLM_91018946936840.

4-layer pre-LN decoder (D=1024, H=16, S=1024, B=4, ff=4096) on 8 NeuronCores.

Sharding: pure token-split data parallel. Each core pair (2b, 2b+1) owns batch
element b; within the pair, core j owns an interleaved half of the sequence at
256-token granularity (j=0: blocks {0,3}, j=1: blocks {1,2} — balanced causal
attention load). Each core keeps its own residual stream for its 512 tokens,
projects K/V for the full sequence locally (duplicated compute, no AllReduce),
and the only communication is one AllToAll per layer exchanging the 512-token
LN1 activations (bf16, issued right after LN1 and overlapped with Q/K/V own
projections). Layer 0's LN1 is computed on the host, so layers need no warmup
exchange. Activations/weights are bf16 with fp32 PSUM accumulation; softmax
runs the exp-then-normalize trick with the denominator riding the AV matmul
as a ones-column in V. All per-core asymmetry (token permutation, causal
masks) lives in input data — the program is SPMD-uniform.
"""
import numpy as np
import ml_dtypes

import concourse.bass as bass
import concourse.mybir as mybir
import concourse.tile as tile
from concourse.bass_utils import run_bass_kernel_spmd
from concourse.vector_clock import ScopedClock

# ---------------------------------------------------------------------------
# Workaround: this walrus build accepts at most ONE semaphore wait per
# instruction ("Too many sync wait commands"). Redistribute Tile-assigned
# waits onto single-wait NoOps in front of the owning instruction, and do the
# same for the kernel-tail drain.
# ---------------------------------------------------------------------------
_MAX_WAITS = 1


def _patched_drain_and_barrier(self, tick_clock, wait_clock):
    nc = self.nc
    probe = nc.sync.nop(hint="drain_waits", nofuse=True)
    wait_clock.add_sem_waits(probe.ins, ScopedClock({None: tick_clock.global_clock}))
    si = probe.ins.sync_info
    waits = list(si.on_wait) if si is not None else []
    probe.ins.sync_info = mybir.SyncInfo(
        on_wait=waits[:_MAX_WAITS],
        on_update=list(si.on_update) if si is not None else [],
    )
    for i in range(_MAX_WAITS, len(waits), _MAX_WAITS):
        extra = nc.sync.nop(hint="drain_waits", nofuse=True)
        extra.ins.sync_info = mybir.SyncInfo(
            on_wait=waits[i : i + _MAX_WAITS], on_update=[])
    nc.sync.drain()
    nc.all_engine_barrier()
    assert self.sems is not None
    popped = nc._tile_sem_poison_stack.pop()
    assert popped is self._sem_poison
    nc.clear_and_free_semaphores(list(self.sems.allocated().values()))
    nc.all_engine_barrier()


_orig_commit = tile.TileContext._commit_instruction


def _patched_commit_instruction(self, inst, lazy_reg_writes=True):
    si = inst.sync_info
    if si is not None and len(si.on_wait) > _MAX_WAITS:
        waits = list(si.on_wait)
        keep, extras = waits[-_MAX_WAITS:], waits[:-_MAX_WAITS]
        engine = inst.engine
        if engine == mybir.EngineType.Unassigned:
            engine = mybir.EngineType.SP
        for w in extras:
            nop = mybir.InstNoOp(
                name=self.nc.get_next_instruction_name(),
                ins=[],
                outs=[],
                engine=engine,
                sync_info=mybir.SyncInfo(on_wait=[w], on_update=[]),
            )
            self._add_instruction(nop)
        inst.sync_info = mybir.SyncInfo(on_wait=keep, on_update=list(si.on_update))
    return _orig_commit(self, inst, lazy_reg_writes)


tile.TileContext._drain_and_barrier = _patched_drain_and_barrier
tile.TileContext._commit_instruction = _patched_commit_instruction

# ---------------------------------------------------------------------------

V, D, H, L, B, S = 32000, 1024, 16, 4, 4, 1024
HD = D // H          # 64
FF = 4 * D           # 4096
EPS = 1e-5
N_CORES = 8
NT = D // 128        # 8   model-dim tiles
NM = FF // 128       # 32  ff tiles
HL = 16              # heads (all, no TP)
OWN = 512            # tokens owned per core
QB = 256             # query block
NKB = S // 128       # 8   key 128-tiles (packed order)

F32 = mybir.dt.float32
F32R = mybir.dt.float32r
BF16 = mybir.dt.bfloat16
FP8 = mybir.dt.float8e4
ADD = mybir.AluOpType.add
MULT = mybir.AluOpType.mult
SUB = mybir.AluOpType.subtract
BYPASS = mybir.AluOpType.bypass
AF = mybir.ActivationFunctionType

REPLICA_GROUPS = [[0, 1], [2, 3], [4, 5], [6, 7]]

# attention tile lists (packed key-128-tile indices); masks supply causality
QB0_TILES_OWN = [0, 1]       # emitted before the exchange lands
QB0_TILES_PEER = [4, 5]      # masked to zero on j=0, real on j=1
QB1_TILES = [0, 1, 2, 3, 4, 5, 6, 7]
N_MASK = 12                  # 4 (qb0) + 8 (qb1) mask rows


def build_nc(repeat=1):
    nc = bass.Bass(trn_type="TRN2", target_bir_lowering=False, debug=False,
                   num_devices=N_CORES)

    def inp(name, shape, dt=F32):
        return nc.dram_tensor(name, list(shape), dt, kind="ExternalInput")

    x0p_d = inp("x0p", [D, OWN])
    h0p_d = inp("h0p", [D, S], BF16)
    wq_d = inp("wq4", [L, NT, 128, NT, 128], BF16)   # host-packed lhsT tiles
    wk_d = inp("wk4", [L, NT, 128, NT, 128], BF16)
    wv_d = inp("wv", [L, D, D], BF16)                # rhs panel layout
    wo_d = inp("wo4", [L, NT, 128, NT, 128], BF16)
    w1_d = inp("w1p", [L, NM, 128, NT, 128], BF16)   # host-packed lhsT tiles
    w2_d = inp("w2p", [L, NT, 128, NM, 128], BF16)
    bq_d = inp("bq", [L, 128, NT])
    bk_d = inp("bk", [L, 128, NT])
    bv_d = inp("bv", [L, 128, NT])
    bo_d = inp("bo", [L, 128, NT])
    b1_d = inp("b1", [L, 128, NM])
    b2_d = inp("b2", [L, 128, NT])
    g1_d = inp("g1", [L, 128, NT])
    be1_d = inp("be1", [L, 128, NT])
    g2_d = inp("g2", [L, 128, NT])
    be2_d = inp("be2", [L, 128, NT])
    gf_d = inp("gf", [128, NT])
    bef_d = inp("bef", [128, NT])
    mask_d = inp("masks", [128, N_MASK, QB])
    ones_d = inp("cones", [128, 128])
    sel_d = inp("selv", [128, 2])

    out_ext = nc.dram_tensor("outt", [D, OWN], F32, kind="ExternalOutput")
    out_v = out_ext.ap().rearrange("(t p) s -> p t s", p=128)

    with tile.TileContext(nc) as tc:
        with (
            nc.allow_low_precision(reason="bf16 weights/activations"),
            tc.tile_pool(name="singles", bufs=1) as singles,
            tc.tile_pool(name="acts", bufs=1) as acts,
            tc.tile_pool(name="wpv", bufs=1) as wpv,     # wv rhs panel
            tc.tile_pool(name="ws1", bufs=6) as ws1,     # lhsT m-tiles
            tc.tile_pool(name="ws2", bufs=2) as ws2,     # w2 m-tiles
            tc.tile_pool(name="wt", bufs=20) as wtp,     # exp'd logits
            tc.tile_pool(name="rows", bufs=2) as rows,
            tc.tile_pool(name="evac", bufs=3) as evac,
            tc.tile_pool(name="pq", bufs=2, space="PSUM") as pq,
            tc.tile_pool(name="pl", bufs=2, space="PSUM") as plp,
            tc.tile_pool(name="pa", bufs=2, space="PSUM") as pap,
            tc.tile_pool(name="pb", bufs=1, space="PSUM") as pbp,
            tc.tile_pool(name="prow", bufs=1, space="PSUM") as prow,
            tc.tile_pool(name="dram", bufs=4, space="DRAM") as dram,
        ):
            # ---- resident constants -------------------------------------
            # (xT's DMA is issued after layer 0's hT load — it isn't read
            # until the WO residual, so it must not delay the first matmuls)
            xT = singles.tile([128, NT, OWN], F32R)
            masks = singles.tile([128, N_MASK, QB], BF16)
            nc.gpsimd.dma_start(out=masks[:], in_=mask_d.ap())
            onesr = singles.tile([128, 128], F32R)
            nc.scalar.dma_start(out=onesr[:],
                                in_=ones_d.ap().bitcast(F32R))
            eps_t = singles.tile([1, 1], F32)
            nc.vector.memset(eps_t[:], EPS)
            selT = singles.tile([128, 2], F32)
            nc.scalar.dma_start(out=selT[:], in_=sel_d.ap())

            def load_pp(d, shape):
                t = singles.tile(list(shape), F32, name=f"pp_{d.name}")
                src = d.ap()
                if len(shape) == 3:
                    src = src.rearrange("l p m -> p l m")
                nc.scalar.dma_start(out=t[:], in_=src)
                return t

            bqT = load_pp(bq_d, [128, L, NT])
            bkT = load_pp(bk_d, [128, L, NT])
            bvT = load_pp(bv_d, [128, L, NT])
            boT = load_pp(bo_d, [128, L, NT])
            b1T = load_pp(b1_d, [128, L, NM])
            b2T = load_pp(b2_d, [128, L, NT])
            g1T = load_pp(g1_d, [128, L, NT])
            be1T = load_pp(be1_d, [128, L, NT])
            g2T = load_pp(g2_d, [128, L, NT])
            be2T = load_pp(be2_d, [128, L, NT])
            gfT = load_pp(gf_d, [128, NT])
            befT = load_pp(bef_d, [128, NT])

            ones_k = onesr[:, 0:1]            # [128,1] lhsT for column sums
            ones_b64 = onesr[0:1, 0:64]       # [1,64] lhsT for row broadcast
            ones_b128 = onesr[0:1, :]         # [1,128] lhsT for row broadcast

            # ---- layernorm over the feature dim of a [128,NT,OWN] tile --
            def layernorm(src, gT, bT, l_idx, dest_fn):
                psx = prow.tile([1, OWN], F32, tag="psx")
                for t in range(NT):
                    nc.tensor.matmul(psx[:], ones_k, src[:, t, :],
                                     start=(t == 0), stop=(t == NT - 1))
                mrow = rows.tile([1, OWN], F32R, tag="mr", bufs=1)
                nc.scalar.mul(out=mrow[:], in_=psx[:], mul=1.0 / D)
                # Σx² reuses the same PSUM bank once the mean is evacuated
                psx2 = prow.tile([1, OWN], F32, tag="psx")
                for t in range(NT):
                    sq = evac.tile([128, OWN], F32R, tag="lntmp", bufs=2)
                    nc.vector.tensor_tensor(out=sq[:], in0=src[:, t, :],
                                            in1=src[:, t, :], op=MULT)
                    nc.tensor.matmul(psx2[:], ones_k, sq[:],
                                     start=(t == 0), stop=(t == NT - 1))
                m2row = rows.tile([1, OWN], F32R, tag="rb")
                nc.vector.tensor_tensor(out=m2row[:], in0=mrow[:],
                                        in1=mrow[:], op=MULT)
                vrow = rows.tile([1, OWN], F32R, tag="rb")
                nc.scalar.mul(out=vrow[:], in_=psx2[:], mul=1.0 / D)
                nc.vector.tensor_tensor(out=vrow[:], in0=vrow[:],
                                        in1=m2row[:], op=SUB)
                srow = rows.tile([1, OWN], F32R, tag="rb")
                nc.scalar.activation(out=srow[:], in_=vrow[:],
                                     func=AF.Sqrt, bias=eps_t[:], scale=1.0)
                rrow = rows.tile([1, OWN], F32R, tag="rrow", bufs=1)
                nc.vector.reciprocal(out=rrow[:], in_=srow[:])
                mrrow = rows.tile([1, OWN], F32R, tag="mr2", bufs=1)
                nc.vector.tensor_tensor(out=mrrow[:], in0=mrow[:],
                                        in1=rrow[:], op=MULT)
                # broadcast the two rows to all partitions via PE matmul +
                # Act evacuation (same pattern as the softmax normalizer;
                # ~4us cheaper than a DRAM round trip)
                bcp = pbp.tile([128, OWN], F32, tag="pb")
                nc.tensor.matmul(bcp[:], ones_b128, rrow[:],
                                 start=True, stop=True)
                prb = evac.tile([128, OWN], F32R, tag="rbb", bufs=1)
                nc.scalar.activation(out=prb[:], in_=bcp[:],
                                     func=AF.Identity, scale=1.0)
                bcp2 = pbp.tile([128, OWN], F32, tag="pb")
                nc.tensor.matmul(bcp2[:], ones_b128, mrrow[:],
                                 start=True, stop=True)
                pmrb = evac.tile([128, OWN], F32R, tag="mrbb", bufs=1)
                nc.scalar.activation(out=pmrb[:], in_=bcp2[:],
                                     func=AF.Identity, scale=1.0)
                for t in range(NT):
                    tmp = evac.tile([128, OWN], F32R, tag="lntmp", bufs=2)
                    nc.vector.tensor_tensor(out=tmp[:], in0=src[:, t, :],
                                            in1=prb[:], op=MULT)
                    nc.vector.tensor_tensor(out=tmp[:], in0=tmp[:],
                                            in1=pmrb[:], op=SUB)
                    if l_idx is not None:
                        gs = gT[:, l_idx, t : t + 1]
                        bs = bT[:, l_idx, t : t + 1]
                    else:
                        gs = gT[:, t : t + 1]
                        bs = bT[:, t : t + 1]
                    dest_fn(t, tmp, gs, bs)

            def wtile(dram_t, l, m):
                """stream one packed lhsT m-tile [128, NT, 128]"""
                w = ws1.tile([128, NT, 128], BF16, tag="w1")
                nc.sync.dma_start(out=w[:], in_=dram_t.ap()[l, m])
                return w

            # attention helper: one (q-block, head, key-tile) logits+exp
            def qk_exp(KT, QT, qb, hh, pk, mpos):
                base = 64 * (hh % 2)
                hp = hh // 2
                pl = plp.tile([128, QB], F32, tag="pl")
                nc.tensor.matmul(
                    pl[:],
                    KT[base:base + 64, hp, pk * 128:(pk + 1) * 128],
                    QT[base:base + 64, hp, qb * QB:(qb + 1) * QB],
                    start=True, stop=True)
                wt = wtp.tile([128, QB], BF16, tag="wt")
                nc.scalar.activation(out=wt[:], in_=pl[:],
                                     func=AF.Exp, scale=0.125)
                nc.vector.tensor_tensor(out=wt[:], in0=wt[:],
                                        in1=masks[:, mpos, :], op=MULT)
                return wt

            def av_start(Vt, qb, hh, tile_list, wt_aps):
                """AV accumulation + reciprocal; normalization is deferred
                two heads so the broadcast matmul never stalls PE."""
                pa = pap.tile([65, QB], F32, tag="pa")
                for i, pk in enumerate(tile_list):
                    nc.tensor.matmul(
                        pa[:], Vt[:, pk, 65 * hh : 65 * hh + 65], wt_aps[i],
                        start=(i == 0), stop=(i == len(tile_list) - 1))
                rec = rows.tile([1, QB], F32R, tag="rec")
                nc.vector.reciprocal(out=rec[:], in_=pa[64:65, :])
                return (pa, rec, qb, hh)

            def av_finish(attnT, state):
                pa, rec, qb, hh = state
                base = 64 * (hh % 2)
                hp = hh // 2
                pb = pbp.tile([64, QB], F32, tag="pb")
                nc.tensor.matmul(pb[:], ones_b64, rec[:], start=True, stop=True)
                pbs = wtp.tile([64, QB], BF16, tag="wt")
                nc.vector.tensor_copy(out=pbs[:], in_=pb[:])
                nc.vector.tensor_tensor(
                    out=attnT[base:base + 64, hp, qb * QB:(qb + 1) * QB],
                    in0=pa[0:64, :], in1=pbs[:], op=MULT)

            # ---- main body ----------------------------------------------
            for rep in range(repeat):
                for l in range(L):
                    hT = acts.tile([128, NT, S], BF16, tag="ht")
                    if l == 0 and rep == 0:
                        # own half first: Q projection only needs cols 0:OWN
                        h0v = h0p_d.ap().rearrange("(t p) s -> p t s", p=128)
                        nc.sync.dma_start(out=hT[:, :, 0:OWN],
                                          in_=h0v[:, :, 0:OWN])
                        nc.sync.dma_start(out=hT[:, :, OWN:S],
                                          in_=h0v[:, :, OWN:S])
                        nc.scalar.dma_start(
                            out=xT[:],
                            in_=x0p_d.ap().rearrange("(t p) s -> p t s", p=128)
                                .bitcast(F32R))
                        at_out = None
                    else:
                        def wr_h(t, tmp, gs, bs, hT=hT):
                            nc.scalar.activation(
                                out=hT[:, t, 0:OWN], in_=tmp[:],
                                func=AF.Identity, bias=bs, scale=gs)
                        layernorm(xT, g1T, be1T, l, wr_h)
                        # exchange LN1 activations in fp8e4m3 (validated
                        # 4.5e-3 end-to-end) — halves the collective time
                        at_in = dram.tile([128, NT, OWN], FP8,
                                          tag="atin", name=f"ati{l}")
                        at_out = dram.tile([2, 128, NT, OWN], FP8,
                                           tag="atout", name=f"ato{l}")
                        for t in range(NT):
                            f8t = evac.tile([128, OWN], FP8, tag="f8", bufs=3)
                            nc.scalar.activation(out=f8t[:],
                                                 in_=hT[:, t, 0:OWN],
                                                 func=AF.Identity, scale=1.0)
                            nc.sync.dma_start(out=at_in[:, t, :], in_=f8t[:])
                        nc.gpsimd.collective_compute(
                            "AllGather", BYPASS, replica_groups=REPLICA_GROUPS,
                            ins=[at_in.opt()], outs=[at_out.opt()])

                    # Q projection (own tokens, full heads)
                    QT = acts.tile([128, NT, OWN], BF16, tag="qt")
                    for m in range(NT):
                        wl = wtile(wq_d, l, m)
                        ps = pq.tile([128, OWN], F32, tag="pq")
                        for k in range(NT):
                            nc.tensor.matmul(
                                ps[:], wl[:, k, :], hT[:, k, 0:OWN],
                                start=(k == 0), stop=(k == NT - 1))
                        nc.vector.tensor_scalar_add(
                            out=QT[:, m, :], in0=ps[:],
                            scalar1=bqT[:, l, m : m + 1])

                    # K/V projections, own half first
                    wvp = wpv.tile([128, NT, D], BF16, tag="wv")
                    nc.sync.dma_start(
                        out=wvp[:],
                        in_=wv_d.ap()[l].rearrange("(t p) m -> p t m", p=128))
                    KT = acts.tile([128, NT, S], BF16, tag="kt")
                    Vt = acts.tile([128, NKB, HL * 65], BF16, tag="vt")
                    nc.vector.memset(Vt[:], 1.0)

                    def k_proj(cs):
                        for m in range(NT):
                            wl = wtile(wk_d, l, m)
                            ps = pq.tile([128, OWN], F32, tag="pq")
                            for k in range(NT):
                                nc.tensor.matmul(
                                    ps[:], wl[:, k, :], hT[:, k, cs],
                                    start=(k == 0), stop=(k == NT - 1))
                            nc.vector.tensor_scalar_add(
                                out=KT[:, m, cs], in0=ps[:],
                                scalar1=bkT[:, l, m : m + 1])

                    def v_proj(mts):
                        for mt in mts:
                            for half in range(2):
                                ps = pq.tile([128, OWN], F32, tag="pq")
                                for k in range(NT):
                                    nc.tensor.matmul(
                                        ps[:],
                                        hT[:, k, mt * 128:(mt + 1) * 128],
                                        wvp[:, k, half * 512:half * 512 + 512],
                                        start=(k == 0), stop=(k == NT - 1))
                                nc.vector.tensor_copy(
                                    out=Vt[:, mt, :]
                                        .rearrange("p (h c) -> p h c", h=HL)
                                        [:, 8 * half:8 * half + 8, 0:64],
                                    in_=ps[:].rearrange("p (h c) -> p h c", h=8))

                    k_proj(slice(0, OWN))
                    v_proj(range(4))

                    # qb0 attention on own key tiles while the exchange flies
                    # (first 8 heads only — bounds live exp'd-logit tiles)
                    wts0 = {}
                    for hh in range(HL // 2):
                        wts0[hh] = [qk_exp(KT, QT, 0, hh, pk, i)
                                    for i, pk in enumerate(QB0_TILES_OWN)]

                    # peer activations land: read back both rank slabs and
                    # select the peer one (per-core 0/1 scalars keep the
                    # program SPMD-uniform).
                    if at_out is not None:
                        for t in range(NT):
                            r0 = evac.tile([128, OWN], FP8, tag="f8", bufs=3)
                            r1 = evac.tile([128, OWN], FP8, tag="f8", bufs=3)
                            nc.sync.dma_start(out=r0[:], in_=at_out[0][:, t, :])
                            nc.sync.dma_start(out=r1[:], in_=at_out[1][:, t, :])
                            tmp = evac.tile([128, OWN], BF16, tag="selt",
                                            bufs=2)
                            nc.vector.tensor_scalar_mul(
                                out=hT[:, t, OWN:S], in0=r0[:],
                                scalar1=selT[:, 0:1])
                            nc.vector.tensor_scalar_mul(
                                out=tmp[:], in0=r1[:],
                                scalar1=selT[:, 1:2])
                            nc.vector.tensor_tensor(
                                out=hT[:, t, OWN:S], in0=hT[:, t, OWN:S],
                                in1=tmp[:], op=ADD)
                    k_proj(slice(OWN, S))
                    v_proj(range(4, 8))

                    # finish qb0, run qb1 (normalization pipelined one head)
                    attnT = acts.tile([128, NT, OWN], BF16, tag="at")
                    pending = None
                    for hh in range(HL):
                        wts = (wts0[hh] if hh < HL // 2 else
                               [qk_exp(KT, QT, 0, hh, pk, i)
                                for i, pk in enumerate(QB0_TILES_OWN)])
                        wts = wts + [
                            qk_exp(KT, QT, 0, hh, pk, 2 + i)
                            for i, pk in enumerate(QB0_TILES_PEER)]
                        st = av_start(Vt, 0, hh,
                                      QB0_TILES_OWN + QB0_TILES_PEER,
                                      [w[:] for w in wts])
                        if pending is not None:
                            av_finish(attnT, pending)
                        pending = st
                    # out-projection of one column half (columns whose
                    # attention is already complete) — one m-tile at a time
                    # so it can interleave with the Act-bound qb1 exp chain
                    def wo_m(m, cs):
                        wl = wtile(wo_d, l, m)
                        ps = pq.tile([128, QB], F32, tag="pq")
                        for k in range(NT):
                            nc.tensor.matmul(
                                ps[:], wl[:, k, :], attnT[:, k, cs],
                                start=(k == 0), stop=(k == NT - 1))
                        po = evac.tile([128, QB], BF16, tag="po")
                        nc.vector.tensor_scalar_add(
                            out=po[:], in0=ps[:],
                            scalar1=boT[:, l, m : m + 1])
                        nc.vector.tensor_tensor(
                            out=xT[:, m, cs], in0=xT[:, m, cs],
                            in1=po[:], op=ADD)

                    # flush qb0 and add the V bias on its columns so WO of
                    # the qb0 half can run during qb1's attention
                    av_finish(attnT, pending)
                    pending = None
                    cs0 = slice(0, QB)
                    for t in range(NT):
                        nc.vector.tensor_scalar_add(
                            out=attnT[:, t, cs0], in0=attnT[:, t, cs0],
                            scalar1=bvT[:, l, t : t + 1])
                    for hh in range(HL):
                        wts = [qk_exp(KT, QT, 1, hh, pk, 4 + i)
                               for i, pk in enumerate(QB1_TILES)]
                        st = av_start(Vt, 1, hh, QB1_TILES,
                                      [w[:] for w in wts])
                        if pending is not None:
                            av_finish(attnT, pending)
                        pending = st
                        if hh % 2 == 1:
                            wo_m(hh // 2, cs0)
                    av_finish(attnT, pending)
                    cs1 = slice(QB, 2 * QB)
                    for t in range(NT):
                        nc.vector.tensor_scalar_add(
                            out=attnT[:, t, cs1], in0=attnT[:, t, cs1],
                            scalar1=bvT[:, l, t : t + 1])
                    for m in range(NT):
                        wo_m(m, cs1)

                    # LN2 -> h2 (reuses QT's slot; QT is dead after qb1)
                    h2 = acts.tile([128, NT, OWN], BF16, tag="qt")

                    def wr_h2(t, tmp, gs, bs, h2=h2):
                        if t % 2 == 0:
                            nc.scalar.activation(
                                out=h2[:, t, :], in_=tmp[:],
                                func=AF.Identity, bias=bs, scale=gs)
                        else:
                            nc.vector.tensor_scalar(
                                out=h2[:, t, :], in0=tmp[:],
                                scalar1=gs, scalar2=bs, op0=MULT, op1=ADD)

                    layernorm(xT, g2T, be2T, l, wr_h2)

                    # MLP
                    gTc = acts.tile([128, NM, OWN], BF16, tag="gt")
                    for m in range(NM):
                        w1t = wtile(w1_d, l, m)
                        ps = pq.tile([128, OWN], F32, tag="pq")
                        for k in range(NT):
                            nc.tensor.matmul(
                                ps[:], w1t[:, k, :], h2[:, k, :],
                                start=(k == 0), stop=(k == NT - 1))
                        nc.scalar.activation(
                            out=gTc[:, m, :], in_=ps[:], func=AF.Gelu,
                            bias=b1T[:, l, m : m + 1], scale=1.0)
                    for m in range(NT):
                        w2t = ws2.tile([128, NM, 128], BF16, tag="w2")
                        nc.sync.dma_start(out=w2t[:], in_=w2_d.ap()[l, m])
                        ps = pq.tile([128, OWN], F32, tag="pq")
                        for k in range(NM):
                            nc.tensor.matmul(
                                ps[:], w2t[:, k, :], gTc[:, k, :],
                                start=(k == 0), stop=(k == NM - 1))
                        po = evac.tile([128, OWN], BF16, tag="po")
                        nc.vector.tensor_scalar_add(
                            out=po[:], in0=ps[:],
                            scalar1=b2T[:, l, m : m + 1])
                        nc.vector.tensor_tensor(
                            out=xT[:, m, :], in0=xT[:, m, :],
                            in1=po[:], op=ADD)

            # final LN -> output
            def wr_out(t, tmp, gs, bs):
                ot = evac.tile([128, OWN], F32, tag="ot", bufs=3)
                nc.scalar.activation(out=ot[:], in_=tmp[:],
                                     func=AF.Identity, bias=bs, scale=gs)
                eng = [nc.sync, nc.scalar, nc.gpsimd][t % 3]
                eng.dma_start(out=out_v[:, t, :], in_=ot[:])

            layernorm(xT, gfT, befT, None, wr_out)

    return nc


# ---------------------------------------------------------------------------
# host side
# ---------------------------------------------------------------------------

def _sinusoidal_pe(s, d):
    pos = np.arange(s, dtype=np.float32)[:, None]
    div = np.exp(np.arange(0, d, 2, dtype=np.float32)
                 * np.float32(-np.log(10000.0) / d)).astype(np.float32)
    pe = np.zeros((s, d), dtype=np.float32)
    pe[:, 0::2] = np.sin(pos * div)
    pe[:, 1::2] = np.cos(pos * div)
    return pe


def _pp128(v):
    """[L?, n*128] -> [L?, 128, n] with feature = 128*m + p."""
    v = np.asarray(v, dtype=np.float32)
    if v.ndim == 1:
        return np.ascontiguousarray(v.reshape(-1, 128).T)
    lq, n = v.shape
    return np.ascontiguousarray(v.reshape(lq, n // 128, 128).transpose(0, 2, 1))


# own-major token permutation per pair-rank j (global token indices)
def _perm(j):
    blocks = [[0, 3, 1, 2], [1, 2, 0, 3]][j]
    return np.concatenate([np.arange(256 * b, 256 * b + 256) for b in blocks])


# packed key-128-tile -> global key-128-tile
def _pk2g(j):
    return [[0, 1, 6, 7, 2, 3, 4, 5], [2, 3, 4, 5, 0, 1, 6, 7]][j]


def _masks(j):
    """[128, N_MASK, QB]: rows 0-3 = qb0 tiles [0,1,4,5]; 4-11 = qb1 tiles."""
    pk2g = _pk2g(j)
    a0 = [0, 1][j]
    a1 = [3, 2][j]
    m = np.zeros((128, N_MASK, QB), dtype=np.float32)
    q = np.arange(QB)

    def fill(row, g, a):
        if g <= 2 * a - 1:
            m[:, row, :] = 1.0
        elif g == 2 * a:
            for p in range(128):
                m[p, row, :] = (q >= p)
        elif g == 2 * a + 1:
            for p in range(128):
                m[p, row, :] = (q >= 128 + p)
        # else stays 0 (fully non-causal)

    for i, pk in enumerate([0, 1, 4, 5]):
        fill(i, pk2g[pk], a0)
    for i, pk in enumerate(range(8)):
        fill(4 + i, pk2g[pk], a1)
    return m


def _ln_np(x, g, b):
    m = x.mean(-1, keepdims=True)
    v = x.var(-1, keepdims=True)
    return (x - m) / np.sqrt(v + EPS) * g + b


_NC_CACHE = {}


def _get_nc(repeat=1):
    if repeat not in _NC_CACHE:
        _NC_CACHE[repeat] = build_nc(repeat)
    return _NC_CACHE[repeat]


def make_in_maps(input_ids, tok_emb, wq, bq, wk, bk, wv, bv, wo, bo,
                 ln1_g, ln1_b, ln2_g, ln2_b, w1, b1, w2, b2, lnf_g, lnf_b):
    input_ids = np.asarray(input_ids)
    pe = _sinusoidal_pe(S, D)
    cones = np.ones((128, 128), dtype=np.float32)
    bfc = lambda a: np.ascontiguousarray(a).astype(ml_dtypes.bfloat16)

    # weights identical on every core
    pack = lambda w, nm: bfc(
        w.reshape(L, NT, 128, nm, 128).transpose(0, 3, 2, 1, 4))
    w1p = pack(w1, NM)
    w2p = bfc(w2.reshape(L, NM, 128, NT, 128).transpose(0, 3, 2, 1, 4))
    shared = {
        "wq4": pack(wq, NT), "wk4": pack(wk, NT), "wv": bfc(wv),
        "wo4": pack(wo, NT),
        "w1p": w1p, "w2p": w2p,
        "bq": _pp128(bq), "bk": _pp128(bk), "bv": _pp128(bv),
        "bo": _pp128(bo), "b1": _pp128(b1), "b2": _pp128(b2),
        "g1": _pp128(ln1_g), "be1": _pp128(ln1_b),
        "g2": _pp128(ln2_g), "be2": _pp128(ln2_b),
        "gf": _pp128(lnf_g), "bef": _pp128(lnf_b),
        "cones": cones,
    }
    perms = [_perm(0), _perm(1)]
    mask_arrs = [_masks(0), _masks(1)]

    in_maps = []
    for core in range(N_CORES):
        b = core // 2
        j = core % 2
        x0 = (tok_emb[input_ids[b]] + pe).astype(np.float32)   # [S, D]
        h0 = _ln_np(x0, ln1_g[0], ln1_b[0])
        m = dict(shared)
        m["x0p"] = np.ascontiguousarray(x0[perms[j][:OWN]].T)
        m["h0p"] = bfc(h0[perms[j]].T)
        m["masks"] = mask_arrs[j]
        sel = np.zeros((128, 2), dtype=np.float32)
        sel[:, 1 - j] = 1.0
        m["selv"] = sel
        in_maps.append(m)
    return in_maps


def kernel(input_ids, attention_mask, tok_emb, ln1_g, ln1_b, wq, bq, wk, bk,
           wv, bv, wo, bo, ln2_g, ln2_b, w1, b1, w2, b2, lnf_g, lnf_b,
           _repeat=1):
    args = [np.asarray(a, dtype=np.float32) for a in
            (tok_emb, wq, bq, wk, bk, wv, bv, wo, bo,
             ln1_g, ln1_b, ln2_g, ln2_b, w1, b1, w2, b2, lnf_g, lnf_b)]
    (tok_emb, wq, bq, wk, bk, wv, bv, wo, bo,
     ln1_g, ln1_b, ln2_g, ln2_b, w1, b1, w2, b2, lnf_g, lnf_b) = args
    in_maps = make_in_maps(input_ids, tok_emb, wq, bq, wk, bk, wv, bv, wo, bo,
                           ln1_g, ln1_b, ln2_g, ln2_b, w1, b1, w2, b2,
                           lnf_g, lnf_b)
    nc = _get_nc(_repeat)
    res = run_bass_kernel_spmd(nc, in_maps, list(range(N_CORES)))
    out = np.empty((B, S, D), dtype=np.float32)
    for core in range(N_CORES):
        b = core // 2
        j = core % 2
        out[b, _perm(j)[:OWN]] = res.results[core]["outt"].T
    return out


# revision 96
# speedup vs baseline: 1.0013x; 1.0013x over previous
"""Trainium2 Bass kernel for nn_DecoderLM_91018946936840.

4-layer pre-LN decoder (D=1024, H=16, S=1024, B=4, ff=4096) on 8 NeuronCores.

Sharding: pure token-split data parallel. Each core pair (2b, 2b+1) owns batch
element b; within the pair, core j owns an interleaved half of the sequence at
256-token granularity (j=0: blocks {0,3}, j=1: blocks {1,2} — balanced causal
attention load). Each core keeps its own residual stream for its 512 tokens,
projects K/V for the full sequence locally (duplicated compute, no AllReduce),
and the only communication is one AllToAll per layer exchanging the 512-token
LN1 activations (bf16, issued right after LN1 and overlapped with Q/K/V own
projections). Layer 0's LN1 is computed on the host, so layers need no warmup
exchange. Activations/weights are bf16 with fp32 PSUM accumulation; softmax
runs the exp-then-normalize trick with the denominator riding the AV matmul
as a ones-column in V. All per-core asymmetry (token permutation, causal
masks) lives in input data — the program is SPMD-uniform.
"""
import numpy as np
import ml_dtypes

import concourse.bass as bass
import concourse.mybir as mybir
import concourse.tile as tile
from concourse.bass_utils import run_bass_kernel_spmd
from concourse.vector_clock import ScopedClock

# ---------------------------------------------------------------------------
# Workaround: this walrus build accepts at most ONE semaphore wait per
# instruction ("Too many sync wait commands"). Redistribute Tile-assigned
# waits onto single-wait NoOps in front of the owning instruction, and do the
# same for the kernel-tail drain.
# ---------------------------------------------------------------------------
_MAX_WAITS = 1


def _patched_drain_and_barrier(self, tick_clock, wait_clock):
    nc = self.nc
    probe = nc.sync.nop(hint="drain_waits", nofuse=True)
    wait_clock.add_sem_waits(probe.ins, ScopedClock({None: tick_clock.global_clock}))
    si = probe.ins.sync_info
    waits = list(si.on_wait) if si is not None else []
    probe.ins.sync_info = mybir.SyncInfo(
        on_wait=waits[:_MAX_WAITS],
        on_update=list(si.on_update) if si is not None else [],
    )
    for i in range(_MAX_WAITS, len(waits), _MAX_WAITS):
        extra = nc.sync.nop(hint="drain_waits", nofuse=True)
        extra.ins.sync_info = mybir.SyncInfo(
            on_wait=waits[i : i + _MAX_WAITS], on_update=[])
    nc.sync.drain()
    nc.all_engine_barrier()
    assert self.sems is not None
    popped = nc._tile_sem_poison_stack.pop()
    assert popped is self._sem_poison
    nc.clear_and_free_semaphores(list(self.sems.allocated().values()))
    nc.all_engine_barrier()


_orig_commit = tile.TileContext._commit_instruction


def _patched_commit_instruction(self, inst, lazy_reg_writes=True):
    si = inst.sync_info
    if si is not None and len(si.on_wait) > _MAX_WAITS:
        waits = list(si.on_wait)
        keep, extras = waits[-_MAX_WAITS:], waits[:-_MAX_WAITS]
        engine = inst.engine
        if engine == mybir.EngineType.Unassigned:
            engine = mybir.EngineType.SP
        for w in extras:
            nop = mybir.InstNoOp(
                name=self.nc.get_next_instruction_name(),
                ins=[],
                outs=[],
                engine=engine,
                sync_info=mybir.SyncInfo(on_wait=[w], on_update=[]),
            )
            self._add_instruction(nop)
        inst.sync_info = mybir.SyncInfo(on_wait=keep, on_update=list(si.on_update))
    return _orig_commit(self, inst, lazy_reg_writes)


tile.TileContext._drain_and_barrier = _patched_drain_and_barrier
tile.TileContext._commit_instruction = _patched_commit_instruction

# ---------------------------------------------------------------------------

V, D, H, L, B, S = 32000, 1024, 16, 4, 4, 1024
HD = D // H          # 64
FF = 4 * D           # 4096
EPS = 1e-5
N_CORES = 8
NT = D // 128        # 8   model-dim tiles
NM = FF // 128       # 32  ff tiles
HL = 16              # heads (all, no TP)
OWN = 512            # tokens owned per core
QB = 256             # query block
NKB = S // 128       # 8   key 128-tiles (packed order)

F32 = mybir.dt.float32
F32R = mybir.dt.float32r
BF16 = mybir.dt.bfloat16
FP8 = mybir.dt.float8e4
ADD = mybir.AluOpType.add
MULT = mybir.AluOpType.mult
SUB = mybir.AluOpType.subtract
BYPASS = mybir.AluOpType.bypass
AF = mybir.ActivationFunctionType

REPLICA_GROUPS = [[0, 1], [2, 3], [4, 5], [6, 7]]

# attention tile lists (packed key-128-tile indices); masks supply causality
QB0_TILES_OWN = [0, 1]       # emitted before the exchange lands
QB0_TILES_PEER = [4, 5]      # masked to zero on j=0, real on j=1
QB1_TILES = [0, 1, 2, 3, 4, 5, 6, 7]
N_MASK = 12                  # 4 (qb0) + 8 (qb1) mask rows


def build_nc(repeat=1):
    nc = bass.Bass(trn_type="TRN2", target_bir_lowering=False, debug=False,
                   num_devices=N_CORES)

    def inp(name, shape, dt=F32):
        return nc.dram_tensor(name, list(shape), dt, kind="ExternalInput")

    x0p_d = inp("x0p", [D, OWN])
    h0p_d = inp("h0p", [D, S], BF16)
    wq_d = inp("wq4", [L, NT, 128, NT, 128], BF16)   # host-packed lhsT tiles
    wk_d = inp("wk4", [L, NT, 128, NT, 128], BF16)
    wv_d = inp("wv", [L, D, D], BF16)                # rhs panel layout
    wo_d = inp("wo4", [L, NT, 128, NT, 128], BF16)
    w1_d = inp("w1p", [L, NM, 128, NT, 128], BF16)   # host-packed lhsT tiles
    w2_d = inp("w2p", [L, NT, 128, NM, 128], BF16)
    bq_d = inp("bq", [L, 128, NT])
    bk_d = inp("bk", [L, 128, NT])
    bv_d = inp("bv", [L, 128, NT])
    bo_d = inp("bo", [L, 128, NT])
    b1_d = inp("b1", [L, 128, NM])
    b2_d = inp("b2", [L, 128, NT])
    g1_d = inp("g1", [L, 128, NT])
    be1_d = inp("be1", [L, 128, NT])
    g2_d = inp("g2", [L, 128, NT])
    be2_d = inp("be2", [L, 128, NT])
    gf_d = inp("gf", [128, NT])
    bef_d = inp("bef", [128, NT])
    mask_d = inp("masks", [128, N_MASK, QB])
    ones_d = inp("cones", [128, 128])
    sel_d = inp("selv", [128, 2])

    out_ext = nc.dram_tensor("outt", [D, OWN], F32, kind="ExternalOutput")
    out_v = out_ext.ap().rearrange("(t p) s -> p t s", p=128)

    with tile.TileContext(nc) as tc:
        with (
            nc.allow_low_precision(reason="bf16 weights/activations"),
            tc.tile_pool(name="singles", bufs=1) as singles,
            tc.tile_pool(name="acts", bufs=1) as acts,
            tc.tile_pool(name="wpv", bufs=1) as wpv,     # wv rhs panel
            tc.tile_pool(name="ws1", bufs=6) as ws1,     # lhsT m-tiles
            tc.tile_pool(name="ws2", bufs=2) as ws2,     # w2 m-tiles
            tc.tile_pool(name="wt", bufs=20) as wtp,     # exp'd logits
            tc.tile_pool(name="rows", bufs=2) as rows,
            tc.tile_pool(name="evac", bufs=3) as evac,
            tc.tile_pool(name="pq", bufs=2, space="PSUM") as pq,
            tc.tile_pool(name="pl", bufs=2, space="PSUM") as plp,
            tc.tile_pool(name="pa", bufs=2, space="PSUM") as pap,
            tc.tile_pool(name="pb", bufs=1, space="PSUM") as pbp,
            tc.tile_pool(name="prow", bufs=1, space="PSUM") as prow,
            tc.tile_pool(name="dram", bufs=4, space="DRAM") as dram,
        ):
            # ---- resident constants -------------------------------------
            # (xT's DMA is issued after layer 0's hT load — it isn't read
            # until the WO residual, so it must not delay the first matmuls)
            xT = singles.tile([128, NT, OWN], F32R)
            masks = singles.tile([128, N_MASK, QB], BF16)
            nc.gpsimd.dma_start(out=masks[:], in_=mask_d.ap())
            onesr = singles.tile([128, 128], F32R)
            nc.scalar.dma_start(out=onesr[:],
                                in_=ones_d.ap().bitcast(F32R))
            eps_t = singles.tile([1, 1], F32)
            nc.vector.memset(eps_t[:], EPS)
            selT = singles.tile([128, 2], F32)
            nc.scalar.dma_start(out=selT[:], in_=sel_d.ap())

            def load_pp(d, shape):
                t = singles.tile(list(shape), F32, name=f"pp_{d.name}")
                src = d.ap()
                if len(shape) == 3:
                    src = src.rearrange("l p m -> p l m")
                nc.scalar.dma_start(out=t[:], in_=src)
                return t

            bqT = load_pp(bq_d, [128, L, NT])
            bkT = load_pp(bk_d, [128, L, NT])
            bvT = load_pp(bv_d, [128, L, NT])
            boT = load_pp(bo_d, [128, L, NT])
            b1T = load_pp(b1_d, [128, L, NM])
            b2T = load_pp(b2_d, [128, L, NT])
            g1T = load_pp(g1_d, [128, L, NT])
            be1T = load_pp(be1_d, [128, L, NT])
            g2T = load_pp(g2_d, [128, L, NT])
            be2T = load_pp(be2_d, [128, L, NT])
            gfT = load_pp(gf_d, [128, NT])
            befT = load_pp(bef_d, [128, NT])

            ones_k = onesr[:, 0:1]            # [128,1] lhsT for column sums
            ones_b64 = onesr[0:1, 0:64]       # [1,64] lhsT for row broadcast
            ones_b128 = onesr[0:1, :]         # [1,128] lhsT for row broadcast

            # ---- layernorm over the feature dim of a [128,NT,OWN] tile --
            def layernorm(src, gT, bT, l_idx, dest_fn):
                psx = prow.tile([1, OWN], F32, tag="psx")
                for t in range(NT):
                    nc.tensor.matmul(psx[:], ones_k, src[:, t, :],
                                     start=(t == 0), stop=(t == NT - 1))
                mrow = rows.tile([1, OWN], F32R, tag="mr", bufs=1)
                nc.scalar.mul(out=mrow[:], in_=psx[:], mul=1.0 / D)
                # Σx² reuses the same PSUM bank once the mean is evacuated
                psx2 = prow.tile([1, OWN], F32, tag="psx")
                for t in range(NT):
                    sq = evac.tile([128, OWN], F32R, tag="lntmp", bufs=2)
                    nc.vector.tensor_tensor(out=sq[:], in0=src[:, t, :],
                                            in1=src[:, t, :], op=MULT)
                    nc.tensor.matmul(psx2[:], ones_k, sq[:],
                                     start=(t == 0), stop=(t == NT - 1))
                m2row = rows.tile([1, OWN], F32R, tag="rb")
                nc.vector.tensor_tensor(out=m2row[:], in0=mrow[:],
                                        in1=mrow[:], op=MULT)
                vrow = rows.tile([1, OWN], F32R, tag="rb")
                nc.scalar.mul(out=vrow[:], in_=psx2[:], mul=1.0 / D)
                nc.vector.tensor_tensor(out=vrow[:], in0=vrow[:],
                                        in1=m2row[:], op=SUB)
                srow = rows.tile([1, OWN], F32R, tag="rb")
                nc.scalar.activation(out=srow[:], in_=vrow[:],
                                     func=AF.Sqrt, bias=eps_t[:], scale=1.0)
                rrow = rows.tile([1, OWN], F32R, tag="rrow", bufs=1)
                nc.vector.reciprocal(out=rrow[:], in_=srow[:])
                mrrow = rows.tile([1, OWN], F32R, tag="mr2", bufs=1)
                nc.vector.tensor_tensor(out=mrrow[:], in0=mrow[:],
                                        in1=rrow[:], op=MULT)
                # broadcast the two rows to all partitions via PE matmul +
                # Act evacuation (same pattern as the softmax normalizer;
                # ~4us cheaper than a DRAM round trip)
                bcp = pbp.tile([128, OWN], F32, tag="pb")
                nc.tensor.matmul(bcp[:], ones_b128, rrow[:],
                                 start=True, stop=True)
                prb = evac.tile([128, OWN], F32R, tag="rbb", bufs=1)
                nc.scalar.activation(out=prb[:], in_=bcp[:],
                                     func=AF.Identity, scale=1.0)
                bcp2 = pbp.tile([128, OWN], F32, tag="pb")
                nc.tensor.matmul(bcp2[:], ones_b128, mrrow[:],
                                 start=True, stop=True)
                pmrb = evac.tile([128, OWN], F32R, tag="mrbb", bufs=1)
                nc.scalar.activation(out=pmrb[:], in_=bcp2[:],
                                     func=AF.Identity, scale=1.0)
                for t in range(NT):
                    tmp = evac.tile([128, OWN], F32R, tag="lntmp", bufs=2)
                    nc.vector.tensor_tensor(out=tmp[:], in0=src[:, t, :],
                                            in1=prb[:], op=MULT)
                    nc.vector.tensor_tensor(out=tmp[:], in0=tmp[:],
                                            in1=pmrb[:], op=SUB)
                    if l_idx is not None:
                        gs = gT[:, l_idx, t : t + 1]
                        bs = bT[:, l_idx, t : t + 1]
                    else:
                        gs = gT[:, t : t + 1]
                        bs = bT[:, t : t + 1]
                    dest_fn(t, tmp, gs, bs)

            def wtile(dram_t, l, m):
                """stream one packed lhsT m-tile [128, NT, 128]"""
                w = ws1.tile([128, NT, 128], BF16, tag="w1")
                nc.sync.dma_start(out=w[:], in_=dram_t.ap()[l, m])
                return w

            # attention helper: one (q-block, head, key-tile) logits+exp
            def qk_exp(KT, QT, qb, hh, pk, mpos):
                base = 64 * (hh % 2)
                hp = hh // 2
                pl = plp.tile([128, QB], F32, tag="pl")
                nc.tensor.matmul(
                    pl[:],
                    KT[base:base + 64, hp, pk * 128:(pk + 1) * 128],
                    QT[base:base + 64, hp, qb * QB:(qb + 1) * QB],
                    start=True, stop=True)
                wt = wtp.tile([128, QB], BF16, tag="wt")
                nc.scalar.activation(out=wt[:], in_=pl[:],
                                     func=AF.Exp, scale=0.125)
                nc.vector.tensor_tensor(out=wt[:], in0=wt[:],
                                        in1=masks[:, mpos, :], op=MULT)
                return wt

            def av_start(Vt, qb, hh, tile_list, wt_aps):
                """AV accumulation + reciprocal; normalization is deferred
                two heads so the broadcast matmul never stalls PE."""
                pa = pap.tile([65, QB], F32, tag="pa")
                for i, pk in enumerate(tile_list):
                    nc.tensor.matmul(
                        pa[:], Vt[:, pk, 65 * hh : 65 * hh + 65], wt_aps[i],
                        start=(i == 0), stop=(i == len(tile_list) - 1))
                rec = rows.tile([1, QB], F32R, tag="rec")
                nc.vector.reciprocal(out=rec[:], in_=pa[64:65, :])
                return (pa, rec, qb, hh)

            def av_finish(attnT, state):
                pa, rec, qb, hh = state
                base = 64 * (hh % 2)
                hp = hh // 2
                pb = pbp.tile([64, QB], F32, tag="pb")
                nc.tensor.matmul(pb[:], ones_b64, rec[:], start=True, stop=True)
                pbs = wtp.tile([64, QB], BF16, tag="wt")
                nc.vector.tensor_copy(out=pbs[:], in_=pb[:])
                nc.vector.tensor_tensor(
                    out=attnT[base:base + 64, hp, qb * QB:(qb + 1) * QB],
                    in0=pa[0:64, :], in1=pbs[:], op=MULT)

            # ---- main body ----------------------------------------------
            for rep in range(repeat):
                for l in range(L):
                    hT = acts.tile([128, NT, S], BF16, tag="ht")
                    if l == 0 and rep == 0:
                        # own half first: Q projection only needs cols 0:OWN
                        h0v = h0p_d.ap().rearrange("(t p) s -> p t s", p=128)
                        nc.sync.dma_start(out=hT[:, :, 0:OWN],
                                          in_=h0v[:, :, 0:OWN])
                        nc.scalar.dma_start(
                            out=xT[:],
                            in_=x0p_d.ap().rearrange("(t p) s -> p t s", p=128)
                                .bitcast(F32R))
                        at_out = None
                    else:
                        def wr_h(t, tmp, gs, bs, hT=hT):
                            nc.scalar.activation(
                                out=hT[:, t, 0:OWN], in_=tmp[:],
                                func=AF.Identity, bias=bs, scale=gs)
                        layernorm(xT, g1T, be1T, l, wr_h)
                        # exchange LN1 activations in fp8e4m3 (validated
                        # 4.5e-3 end-to-end) — halves the collective time
                        at_in = dram.tile([128, NT, OWN], FP8,
                                          tag="atin", name=f"ati{l}")
                        at_out = dram.tile([2, 128, NT, OWN], FP8,
                                           tag="atout", name=f"ato{l}")
                        for t in range(NT):
                            f8t = evac.tile([128, OWN], FP8, tag="f8", bufs=3)
                            nc.scalar.activation(out=f8t[:],
                                                 in_=hT[:, t, 0:OWN],
                                                 func=AF.Identity, scale=1.0)
                            nc.sync.dma_start(out=at_in[:, t, :], in_=f8t[:])
                        nc.gpsimd.collective_compute(
                            "AllGather", BYPASS, replica_groups=REPLICA_GROUPS,
                            ins=[at_in.opt()], outs=[at_out.opt()])

                    # Q projection (own tokens, full heads)
                    QT = acts.tile([128, NT, OWN], BF16, tag="qt")
                    for m in range(NT):
                        wl = wtile(wq_d, l, m)
                        ps = pq.tile([128, OWN], F32, tag="pq")
                        for k in range(NT):
                            nc.tensor.matmul(
                                ps[:], wl[:, k, :], hT[:, k, 0:OWN],
                                start=(k == 0), stop=(k == NT - 1))
                        nc.vector.tensor_scalar_add(
                            out=QT[:, m, :], in0=ps[:],
                            scalar1=bqT[:, l, m : m + 1])

                    # K/V projections, own half first
                    wvp = wpv.tile([128, NT, D], BF16, tag="wv")
                    nc.sync.dma_start(
                        out=wvp[:],
                        in_=wv_d.ap()[l].rearrange("(t p) m -> p t m", p=128))
                    KT = acts.tile([128, NT, S], BF16, tag="kt")
                    Vt = acts.tile([128, NKB, HL * 65], BF16, tag="vt")
                    nc.vector.memset(Vt[:], 1.0)

                    def k_proj(cs):
                        for m in range(NT):
                            wl = wtile(wk_d, l, m)
                            ps = pq.tile([128, OWN], F32, tag="pq")
                            for k in range(NT):
                                nc.tensor.matmul(
                                    ps[:], wl[:, k, :], hT[:, k, cs],
                                    start=(k == 0), stop=(k == NT - 1))
                            nc.vector.tensor_scalar_add(
                                out=KT[:, m, cs], in0=ps[:],
                                scalar1=bkT[:, l, m : m + 1])

                    def v_proj(mts):
                        for mt in mts:
                            for half in range(2):
                                ps = pq.tile([128, OWN], F32, tag="pq")
                                for k in range(NT):
                                    nc.tensor.matmul(
                                        ps[:],
                                        hT[:, k, mt * 128:(mt + 1) * 128],
                                        wvp[:, k, half * 512:half * 512 + 512],
                                        start=(k == 0), stop=(k == NT - 1))
                                nc.vector.tensor_copy(
                                    out=Vt[:, mt, :]
                                        .rearrange("p (h c) -> p h c", h=HL)
                                        [:, 8 * half:8 * half + 8, 0:64],
                                    in_=ps[:].rearrange("p (h c) -> p h c", h=8))

                    k_proj(slice(0, OWN))
                    v_proj(range(4))
                    if at_out is None and rep == 0:
                        nc.sync.dma_start(
                            out=hT[:, :, OWN:S],
                            in_=h0p_d.ap().rearrange("(t p) s -> p t s",
                                                     p=128)[:, :, OWN:S])

                    # qb0 attention on own key tiles while the exchange flies
                    # (first 8 heads only — bounds live exp'd-logit tiles)
                    wts0 = {}
                    for hh in range(HL // 2):
                        wts0[hh] = [qk_exp(KT, QT, 0, hh, pk, i)
                                    for i, pk in enumerate(QB0_TILES_OWN)]

                    # peer activations land: read back both rank slabs and
                    # select the peer one (per-core 0/1 scalars keep the
                    # program SPMD-uniform).
                    if at_out is not None:
                        for t in range(NT):
                            r0 = evac.tile([128, OWN], FP8, tag="f8", bufs=3)
                            r1 = evac.tile([128, OWN], FP8, tag="f8", bufs=3)
                            nc.sync.dma_start(out=r0[:], in_=at_out[0][:, t, :])
                            nc.sync.dma_start(out=r1[:], in_=at_out[1][:, t, :])
                            tmp = evac.tile([128, OWN], BF16, tag="selt",
                                            bufs=2)
                            nc.vector.tensor_scalar_mul(
                                out=hT[:, t, OWN:S], in0=r0[:],
                                scalar1=selT[:, 0:1])
                            nc.vector.tensor_scalar_mul(
                                out=tmp[:], in0=r1[:],
                                scalar1=selT[:, 1:2])
                            nc.vector.tensor_tensor(
                                out=hT[:, t, OWN:S], in0=hT[:, t, OWN:S],
                                in1=tmp[:], op=ADD)
                    k_proj(slice(OWN, S))
                    v_proj(range(4, 8))

                    # finish qb0, run qb1 (normalization pipelined one head)
                    attnT = acts.tile([128, NT, OWN], BF16, tag="at")
                    pending = None
                    for hh in range(HL):
                        wts = (wts0[hh] if hh < HL // 2 else
                               [qk_exp(KT, QT, 0, hh, pk, i)
                                for i, pk in enumerate(QB0_TILES_OWN)])
                        wts = wts + [
                            qk_exp(KT, QT, 0, hh, pk, 2 + i)
                            for i, pk in enumerate(QB0_TILES_PEER)]
                        st = av_start(Vt, 0, hh,
                                      QB0_TILES_OWN + QB0_TILES_PEER,
                                      [w[:] for w in wts])
                        if pending is not None:
                            av_finish(attnT, pending)
                        pending = st
                    # out-projection of one column half (columns whose
                    # attention is already complete) — one m-tile at a time
                    # so it can interleave with the Act-bound qb1 exp chain
                    def wo_m(m, cs):
                        wl = wtile(wo_d, l, m)
                        ps = pq.tile([128, QB], F32, tag="pq")
                        for k in range(NT):
                            nc.tensor.matmul(
                                ps[:], wl[:, k, :], attnT[:, k, cs],
                                start=(k == 0), stop=(k == NT - 1))
                        po = evac.tile([128, QB], BF16, tag="po")
                        nc.vector.tensor_scalar_add(
                            out=po[:], in0=ps[:],
                            scalar1=boT[:, l, m : m + 1])
                        nc.vector.tensor_tensor(
                            out=xT[:, m, cs], in0=xT[:, m, cs],
                            in1=po[:], op=ADD)

                    # flush qb0 and add the V bias on its columns so WO of
                    # the qb0 half can run during qb1's attention
                    av_finish(attnT, pending)
                    pending = None
                    cs0 = slice(0, QB)
                    for t in range(NT):
                        nc.vector.tensor_scalar_add(
                            out=attnT[:, t, cs0], in0=attnT[:, t, cs0],
                            scalar1=bvT[:, l, t : t + 1])
                    for hh in range(HL):
                        wts = [qk_exp(KT, QT, 1, hh, pk, 4 + i)
                               for i, pk in enumerate(QB1_TILES)]
                        st = av_start(Vt, 1, hh, QB1_TILES,
                                      [w[:] for w in wts])
                        if pending is not None:
                            av_finish(attnT, pending)
                        pending = st
                        if hh % 2 == 1:
                            wo_m(hh // 2, cs0)
                    av_finish(attnT, pending)
                    cs1 = slice(QB, 2 * QB)
                    for t in range(NT):
                        nc.vector.tensor_scalar_add(
                            out=attnT[:, t, cs1], in0=attnT[:, t, cs1],
                            scalar1=bvT[:, l, t : t + 1])
                    for m in range(NT):
                        wo_m(m, cs1)

                    # LN2 -> h2 (reuses QT's slot; QT is dead after qb1)
                    h2 = acts.tile([128, NT, OWN], BF16, tag="qt")

                    def wr_h2(t, tmp, gs, bs, h2=h2):
                        if t % 2 == 0:
                            nc.scalar.activation(
                                out=h2[:, t, :], in_=tmp[:],
                                func=AF.Identity, bias=bs, scale=gs)
                        else:
                            nc.vector.tensor_scalar(
                                out=h2[:, t, :], in0=tmp[:],
                                scalar1=gs, scalar2=bs, op0=MULT, op1=ADD)

                    layernorm(xT, g2T, be2T, l, wr_h2)

                    # MLP
                    gTc = acts.tile([128, NM, OWN], BF16, tag="gt")
                    for m in range(NM):
                        w1t = wtile(w1_d, l, m)
                        ps = pq.tile([128, OWN], F32, tag="pq")
                        for k in range(NT):
                            nc.tensor.matmul(
                                ps[:], w1t[:, k, :], h2[:, k, :],
                                start=(k == 0), stop=(k == NT - 1))
                        nc.scalar.activation(
                            out=gTc[:, m, :], in_=ps[:], func=AF.Gelu,
                            bias=b1T[:, l, m : m + 1], scale=1.0)
                    for m in range(NT):
                        w2t = ws2.tile([128, NM, 128], BF16, tag="w2")
                        nc.sync.dma_start(out=w2t[:], in_=w2_d.ap()[l, m])
                        ps = pq.tile([128, OWN], F32, tag="pq")
                        for k in range(NM):
                            nc.tensor.matmul(
                                ps[:], w2t[:, k, :], gTc[:, k, :],
                                start=(k == 0), stop=(k == NM - 1))
                        po = evac.tile([128, OWN], BF16, tag="po")
                        nc.vector.tensor_scalar_add(
                            out=po[:], in0=ps[:],
                            scalar1=b2T[:, l, m : m + 1])
                        nc.vector.tensor_tensor(
                            out=xT[:, m, :], in0=xT[:, m, :],
                            in1=po[:], op=ADD)

            # final LN -> output
            def wr_out(t, tmp, gs, bs):
                ot = evac.tile([128, OWN], F32, tag="ot", bufs=3)
                nc.scalar.activation(out=ot[:], in_=tmp[:],
                                     func=AF.Identity, bias=bs, scale=gs)
                eng = [nc.sync, nc.scalar, nc.gpsimd][t % 3]
                eng.dma_start(out=out_v[:, t, :], in_=ot[:])

            layernorm(xT, gfT, befT, None, wr_out)

    return nc


# ---------------------------------------------------------------------------
# host side
# ---------------------------------------------------------------------------

def _sinusoidal_pe(s, d):
    pos = np.arange(s, dtype=np.float32)[:, None]
    div = np.exp(np.arange(0, d, 2, dtype=np.float32)
                 * np.float32(-np.log(10000.0) / d)).astype(np.float32)
    pe = np.zeros((s, d), dtype=np.float32)
    pe[:, 0::2] = np.sin(pos * div)
    pe[:, 1::2] = np.cos(pos * div)
    return pe


def _pp128(v):
    """[L?, n*128] -> [L?, 128, n] with feature = 128*m + p."""
    v = np.asarray(v, dtype=np.float32)
    if v.ndim == 1:
        return np.ascontiguousarray(v.reshape(-1, 128).T)
    lq, n = v.shape
    return np.ascontiguousarray(v.reshape(lq, n // 128, 128).transpose(0, 2, 1))


# own-major token permutation per pair-rank j (global token indices)
def _perm(j):
    blocks = [[0, 3, 1, 2], [1, 2, 0, 3]][j]
    return np.concatenate([np.arange(256 * b, 256 * b + 256) for b in blocks])


# packed key-128-tile -> global key-128-tile
def _pk2g(j):
    return [[0, 1, 6, 7, 2, 3, 4, 5], [2, 3, 4, 5, 0, 1, 6, 7]][j]


def _masks(j):
    """[128, N_MASK, QB]: rows 0-3 = qb0 tiles [0,1,4,5]; 4-11 = qb1 tiles."""
    pk2g = _pk2g(j)
    a0 = [0, 1][j]
    a1 = [3, 2][j]
    m = np.zeros((128, N_MASK, QB), dtype=np.float32)
    q = np.arange(QB)

    def fill(row, g, a):
        if g <= 2 * a - 1:
            m[:, row, :] = 1.0
        elif g == 2 * a:
            for p in range(128):
                m[p, row, :] = (q >= p)
        elif g == 2 * a + 1:
            for p in range(128):
                m[p, row, :] = (q >= 128 + p)
        # else stays 0 (fully non-causal)

    for i, pk in enumerate([0, 1, 4, 5]):
        fill(i, pk2g[pk], a0)
    for i, pk in enumerate(range(8)):
        fill(4 + i, pk2g[pk], a1)
    return m


def _ln_np(x, g, b):
    m = x.mean(-1, keepdims=True)
    v = x.var(-1, keepdims=True)
    return (x - m) / np.sqrt(v + EPS) * g + b


_NC_CACHE = {}


def _get_nc(repeat=1):
    if repeat not in _NC_CACHE:
        _NC_CACHE[repeat] = build_nc(repeat)
    return _NC_CACHE[repeat]


def make_in_maps(input_ids, tok_emb, wq, bq, wk, bk, wv, bv, wo, bo,
                 ln1_g, ln1_b, ln2_g, ln2_b, w1, b1, w2, b2, lnf_g, lnf_b):
    input_ids = np.asarray(input_ids)
    pe = _sinusoidal_pe(S, D)
    cones = np.ones((128, 128), dtype=np.float32)
    bfc = lambda a: np.ascontiguousarray(a).astype(ml_dtypes.bfloat16)

    # weights identical on every core
    pack = lambda w, nm: bfc(
        w.reshape(L, NT, 128, nm, 128).transpose(0, 3, 2, 1, 4))
    w1p = pack(w1, NM)
    w2p = bfc(w2.reshape(L, NM, 128, NT, 128).transpose(0, 3, 2, 1, 4))
    shared = {
        "wq4": pack(wq, NT), "wk4": pack(wk, NT), "wv": bfc(wv),
        "wo4": pack(wo, NT),
        "w1p": w1p, "w2p": w2p,
        "bq": _pp128(bq), "bk": _pp128(bk), "bv": _pp128(bv),
        "bo": _pp128(bo), "b1": _pp128(b1), "b2": _pp128(b2),
        "g1": _pp128(ln1_g), "be1": _pp128(ln1_b),
        "g2": _pp128(ln2_g), "be2": _pp128(ln2_b),
        "gf": _pp128(lnf_g), "bef": _pp128(lnf_b),
        "cones": cones,
    }
    perms = [_perm(0), _perm(1)]
    mask_arrs = [_masks(0), _masks(1)]

    in_maps = []
    for core in range(N_CORES):
        b = core // 2
        j = core % 2
        x0 = (tok_emb[input_ids[b]] + pe).astype(np.float32)   # [S, D]
        h0 = _ln_np(x0, ln1_g[0], ln1_b[0])
        m = dict(shared)
        m["x0p"] = np.ascontiguousarray(x0[perms[j][:OWN]].T)
        m["h0p"] = bfc(h0[perms[j]].T)
        m["masks"] = mask_arrs[j]
        sel = np.zeros((128, 2), dtype=np.float32)
        sel[:, 1 - j] = 1.0
        m["selv"] = sel
        in_maps.append(m)
    return in_maps


def kernel(input_ids, attention_mask, tok_emb, ln1_g, ln1_b, wq, bq, wk, bk,
           wv, bv, wo, bo, ln2_g, ln2_b, w1, b1, w2, b2, lnf_g, lnf_b,
           _repeat=1):
    args = [np.asarray(a, dtype=np.float32) for a in
            (tok_emb, wq, bq, wk, bk, wv, bv, wo, bo,
             ln1_g, ln1_b, ln2_g, ln2_b, w1, b1, w2, b2, lnf_g, lnf_b)]
    (tok_emb, wq, bq, wk, bk, wv, bv, wo, bo,
     ln1_g, ln1_b, ln2_g, ln2_b, w1, b1, w2, b2, lnf_g, lnf_b) = args
    in_maps = make_in_maps(input_ids, tok_emb, wq, bq, wk, bk, wv, bv, wo, bo,
                           ln1_g, ln1_b, ln2_g, ln2_b, w1, b1, w2, b2,
                           lnf_g, lnf_b)
    nc = _get_nc(_repeat)
    res = run_bass_kernel_spmd(nc, in_maps, list(range(N_CORES)))
    out = np.empty((B, S, D), dtype=np.float32)
    for core in range(N_CORES):
        b = core // 2
        j = core % 2
        out[b, _perm(j)[:OWN]] = res.results[core]["outt"].T
    return out
